# revision 12
# baseline (speedup 1.0000x reference)
"""Cross-attention (B=4, N=2048, C=768, H=12, HD=64) on 8 TRN2 NeuronCores.

Sharding: core = (batch, head_group): 4 batches x 2 groups of 6 heads.
Each core computes its group's Q/K/V projections, per-head-dim LayerNorm,
attention, and a partial output projection; the host sums the two group
partials per batch and adds the bias.

Key optimizations over the fp32 baseline:
 - All matmul operands are fp16 (1 cycle/row on the PE vs 4 for fp32 mode,
   and fast-weight-load applies).  PSUM accumulation stays fp32.
 - Query-token compaction: the reference masks along QUERY rows only, and
   every masked row produces the same output (the uniform average of V,
   since softmax(-1e9 * 1) is uniform).  The host gathers the ~50% unmasked
   tokens per batch, zero-pads to MQ=1152 columns, and scatters back; one
   guaranteed-pad column (q~ = 0 -> scores 0 -> uniform softmax) supplies
   the shared masked-row output.  Attention/exp/out-proj work drops ~2x.
 - Software pipelining: scores (PE) -> exp (ACT) -> PV (PE) run with one
   k-group of lookahead so the ACT exp stream never starves; Q-projection
   and out-projection work is sliced into small "filler" units emitted
   between attention steps to hide it under the ACT-bound window.
 - Softmax denominators come free from a ones-column appended to V; no
   row-max is needed (LN bounds |S| < ~6, exp(S) fits fp16 comfortably).
 - LN scale (HD^-0.5) is folded into the rsqrt via the activation bias:
   exp(-0.5*ln(var+eps) + ln(scale)) = scale * rsqrt(var+eps).
"""

import numpy as np

import concourse.bass as bass
import concourse.mybir as mybir
from concourse import tile
from concourse import bass_utils
from concourse.tile_scheduler import N_PROCS
from concourse.vector_clock import ScopedClock, VectorClock

F32 = mybir.dt.float32
F16 = mybir.dt.float16
AF = mybir.ActivationFunctionType
OP = mybir.AluOpType

B, N, C, H, HD = 4, 2048, 768, 12, 64
G = 2                 # head groups (tensor parallel)
HPG = H // G          # 6 heads per group
CL = HPG * HD         # 384 local channels
P = 128
CH = 512              # token chunk
NCH = N // CH         # 4 k-side chunks
NT = CL // P          # 3 output tiles per group
CT = C // P           # 6 contraction tiles
TT = N // P           # 16 k token tiles
KT_GRP = 2            # k-tiles per exp group
MQ = 1152             # padded compacted q tokens (counts are ~1024+-45)
QCHUNKS = [(0, 512), (512, 512), (1024, 128)]
EPS = 1e-5
SCALE = HD ** -0.5
LNSCALE = float(np.log(SCALE))
NCORES = 8

_nop_ctr = [0]


class _FixedTileContext(tile.TileContext):
    """Workaround for a walrus build that allows at most ONE sync-wait per
    instruction: split multi-wait instructions into single-wait NoOps on the
    same engine, and emit the kernel-tail drain's waits as a nop chain."""

    def _split_multiwait(self, insts):
        out = []
        for inst in insts:
            si = getattr(inst, "sync_info", None)
            waits = list(si.on_wait) if si is not None and si.on_wait else []
            if len(waits) > 1:
                eng = inst.engine
                for w in waits[:-1]:
                    _nop_ctr[0] += 1
                    nop = mybir.InstNoOp(
                        name=f"I-waitsplit-{_nop_ctr[0]}", ins=[], outs=[]
                    )
                    nop.engine = eng
                    nop.sync_info = mybir.SyncInfo(on_wait=[w], on_update=[])
                    self.nc.register_instruction(nop)
                    out.append(nop)
                inst.sync_info = mybir.SyncInfo(
                    on_wait=[waits[-1]], on_update=list(si.on_update)
                )
            out.append(inst)
        return out

    def _lower_ordered_insts(self, ordered):
        ordered = {bb: self._split_multiwait(ins) for bb, ins in ordered.items()}
        super()._lower_ordered_insts(ordered)

    def _drain_and_barrier(self, tick_clock, wait_clock):
        gc = tick_clock.global_clock
        vals = [gc[p] for p in range(N_PROCS)]
        for p in [q for q, v in enumerate(vals) if v > 0]:
            partial = VectorClock(
                [vals[q] if q == p else 0 for q in range(N_PROCS)]
            )
            nop = self.nc.sync.nop(nofuse=True, hint="tail_drain_wait")
            wait_clock.add_sem_waits(nop.ins, ScopedClock({None: partial}))
        self.nc.sync.drain()
        self.nc.all_engine_barrier()
        assert self.sems is not None
        popped = self.nc._tile_sem_poison_stack.pop()
        assert popped is self._sem_poison
        self.nc.clear_and_free_semaphores(list(self.sems.allocated().values()))
        self.nc.all_engine_barrier()


def _mm(nc, out, lhsT, rhs, start, stop):
    nc.tensor.matmul(
        out, lhsT, rhs, start=start, stop=stop, skip_group_check=True
    )


def _body(tc, aps):
    nc = tc.nc
    qxT, kvxT, wq, wk, wv, wp, colsel, bcast, outT = aps

    cpool = tc.alloc_tile_pool(name="consts", bufs=1)
    bpool = tc.alloc_tile_pool(name="big", bufs=1)
    w_pool = tc.alloc_tile_pool(name="wts", bufs=1)

    colsel_sb = cpool.tile([P, NT, HPG], F16, name="colsel", tag="colsel")
    nc.sync.dma_start(colsel_sb[:], colsel[:])
    bcast_sb = cpool.tile([HPG, NT, P], F16, name="bcast", tag="bcast")
    nc.sync.dma_start(bcast_sb[:], bcast[:])
    eps_sb = cpool.tile([HPG, 1], F32, name="eps", tag="eps")
    nc.vector.memset(eps_sb[:], EPS)
    lnq_sb = cpool.tile([HPG, 1], F32, name="lnq", tag="lnq")
    nc.vector.memset(lnq_sb[:], LNSCALE)

    wk_sb = w_pool.tile([P, CT, CL], F16, name="wk", tag="wk")
    nc.sync.dma_start(wk_sb[:], wk.rearrange("(ct p) m -> p ct m", p=P))
    wv_sb = w_pool.tile([P, CT, CL], F16, name="wv", tag="wv")
    nc.sync.dma_start(wv_sb[:], wv.rearrange("(ct p) m -> p ct m", p=P))
    wq_sb = w_pool.tile([P, CT, CL], F16, name="wq", tag="wq")
    wp_sb = w_pool.tile([P, NT, C], F16, name="wp", tag="wp")

    q_sb = [bpool.tile([P, MQ], F16, name=f"q{t}", tag=f"q{t}")
            for t in range(NT)]
    k_sb = [bpool.tile([P, N], F16, name=f"k{t}", tag=f"k{t}")
            for t in range(NT)]
    v_sb = bpool.tile([P, TT, HPG, HD + 1], F16, name="v", tag="v")
    nc.vector.memset(v_sb[:, :, :, HD], 1.0)
    # attention accumulator across the two k-halves (f32, row HD = denom)
    o_acc = bpool.tile([HD + 1, len(QCHUNKS), HPG, CH], F32,
                       name="oacc", tag="oacc")

    sq_pool = tc.alloc_tile_pool(name="sq", bufs=6)
    st32 = tc.alloc_tile_pool(name="st32", bufs=6)
    st16 = tc.alloc_tile_pool(name="st16", bufs=4)
    xkv_pool = tc.alloc_tile_pool(name="xkv", bufs=12)
    xq_pool = tc.alloc_tile_pool(name="xq", bufs=12)

    def chunk_units(xT, w_sb, dst, is_q, coff, W, xpool, pp_pool, st_pool,
                    rr_pool, with_v, use_act, tags):
        """Emission units for projecting one token chunk + per-head-dim LN
        (+ V projection).  use_act routes squares/copies to the Scalar
        engine (good when exp isn't running) vs Vector."""
        tag_pp, tag_st, tag_rr = tags
        st = {"xts": [], "sqs": [None] * NT}
        units = []

        def dmas():
            for ct in range(CT):
                xt = xpool.tile([P, CH], F16, name="xt", tag="xt")
                nc.sync.dma_start(
                    xt[:, 0:W], xT[ct * P:(ct + 1) * P, coff:coff + W])
                st["xts"].append(xt)
        units.append(dmas)
        for t in range(NT):
            def proj(t=t):
                pp = pp_pool.tile([P, CH], F32, name="pp", tag=tag_pp)
                for ct in range(CT):
                    _mm(nc, pp[:, 0:W], w_sb[:, ct, t * P:(t + 1) * P],
                        st["xts"][ct][:, 0:W], ct == 0, ct == CT - 1)
                nc.vector.tensor_copy(dst[t][:, coff:coff + W], pp[:, 0:W])
                sq = sq_pool.tile([P, CH], F16, name="sq", tag="sq")
                if use_act:
                    nc.scalar.activation(sq[:, 0:W], pp[:, 0:W], AF.Square)
                else:
                    nc.vector.tensor_tensor(
                        sq[:, 0:W], dst[t][:, coff:coff + W],
                        dst[t][:, coff:coff + W], OP.mult)
                st["sqs"][t] = sq
            units.append(proj)

        def stats():
            mu_ps = st_pool.tile([HPG, CH], F32, name="mu_ps", tag=tag_st)
            for t in range(NT):
                _mm(nc, mu_ps[:, 0:W], colsel_sb[:, t, :],
                    dst[t][:, coff:coff + W], t == 0, t == NT - 1)
            ms_ps = st_pool.tile([HPG, CH], F32, name="ms_ps", tag=tag_st)
            for t in range(NT):
                _mm(nc, ms_ps[:, 0:W], colsel_sb[:, t, :],
                    st["sqs"][t][:, 0:W], t == 0, t == NT - 1)
            mu = st32.tile([HPG, CH], F32, name="mu", tag="mu")
            nc.vector.tensor_copy(mu[:, 0:W], mu_ps[:, 0:W])
            var = st32.tile([HPG, CH], F32, name="var", tag="var")
            nc.vector.scalar_tensor_tensor(
                var[:, 0:W], mu[:, 0:W], 1.0, mu[:, 0:W], OP.mult, OP.mult)
            nc.vector.tensor_tensor(
                var[:, 0:W], ms_ps[:, 0:W], var[:, 0:W], OP.subtract)
            lnv = st32.tile([HPG, CH], F32, name="lnv", tag="lnv")
            nc.scalar.activation(lnv[:, 0:W], var[:, 0:W], AF.Ln,
                                 bias=eps_sb[:])
            rs = st16.tile([HPG, CH], F16, name="rs", tag="rs")
            if is_q:
                nc.scalar.activation(rs[:, 0:W], lnv[:, 0:W], AF.Exp,
                                     scale=-0.5, bias=lnq_sb[:])
            else:
                nc.scalar.activation(rs[:, 0:W], lnv[:, 0:W], AF.Exp,
                                     scale=-0.5)
            murs = st16.tile([HPG, CH], F16, name="murs", tag="murs")
            nc.vector.scalar_tensor_tensor(
                murs[:, 0:W], mu[:, 0:W], -1.0, rs[:, 0:W],
                OP.mult, OP.mult)
            st["rs"], st["murs"] = rs, murs
        units.append(stats)
        for t in range(NT):
            def apply(t=t):
                rrep = rr_pool.tile([P, CH], F32, name="rrep", tag=tag_rr)
                _mm(nc, rrep[:, 0:W], bcast_sb[:, t, :], st["rs"][:, 0:W],
                    True, True)
                mrep = rr_pool.tile([P, CH], F32, name="mrep", tag=tag_rr)
                _mm(nc, mrep[:, 0:W], bcast_sb[:, t, :], st["murs"][:, 0:W],
                    True, True)
                nc.vector.tensor_tensor(
                    dst[t][:, coff:coff + W], dst[t][:, coff:coff + W],
                    rrep[:, 0:W], OP.mult)
                nc.vector.tensor_tensor(
                    dst[t][:, coff:coff + W], dst[t][:, coff:coff + W],
                    mrep[:, 0:W], OP.add)
            units.append(apply)
        if with_v:
            for tl in range(CH // P):
                def vproj(tl=tl):
                    tt = coff // P + tl
                    vp = pp_pool.tile([P, CH], F32, name="vp", tag=tag_pp)
                    for ct in range(CT):
                        _mm(nc, vp[:, 0:CL],
                            st["xts"][ct][:, tl * P:(tl + 1) * P],
                            wv_sb[:, ct, :], ct == 0, ct == CT - 1)
                    vap = vp[:, 0:CL].rearrange("p (h d) -> p h d", h=HPG)
                    if use_act:
                        nc.scalar.activation(
                            v_sb[:, tt, :, 0:HD], vap, AF.Copy)
                    else:
                        nc.vector.tensor_copy(v_sb[:, tt, :, 0:HD], vap)
                units.append(vproj)
        return units

    # ---------------- pre-phase: k chunks 0-1 + q chunk 0 --------------
    ppA = tc.alloc_tile_pool(name="ppA", bufs=4, space="PSUM")
    stA = tc.alloc_tile_pool(name="stA", bufs=2, space="PSUM")
    rrA = tc.alloc_tile_pool(name="rrA", bufs=2, space="PSUM")

    def kchunk_units(c, use_act, pools, tags):
        return chunk_units(kvxT, wk_sb, k_sb, False, c * CH, CH, xkv_pool,
                           pools[0], pools[1], pools[2], True, use_act, tags)

    def qln_units(qc, use_act, pools, tags):
        coff, W = QCHUNKS[qc]
        return chunk_units(qxT, wq_sb, q_sb, True, coff, W, xq_pool,
                           pools[0], pools[1], pools[2], False, use_act,
                           tags)

    poolsA, tagsA = (ppA, stA, rrA), ("pp", "stp", "rr")
    for u in kchunk_units(0, True, poolsA, tagsA):
        u()
    nc.sync.dma_start(wq_sb[:], wq.rearrange("(ct p) m -> p ct m", p=P))
    nc.sync.dma_start(wp_sb[:], wp.rearrange("(t p) m -> p t m", p=P))
    for u in kchunk_units(1, True, poolsA, tagsA):
        u()
    for u in qln_units(0, True, poolsA, tagsA):
        u()

    for pool in (rrA, stA, ppA):
        pool.release()

    # ---------------- attention pools ----------------------------------
    ps_sp = tc.alloc_tile_pool(name="ps_sp", bufs=2, space="PSUM")
    ps_po = tc.alloc_tile_pool(name="ps_po", bufs=2, space="PSUM")
    ps_mi = tc.alloc_tile_pool(name="ps_mi", bufs=2, space="PSUM")
    e_pool = tc.alloc_tile_pool(name="e", bufs=4)
    o_pool = tc.alloc_tile_pool(name="o", bufs=6)
    den_pool = tc.alloc_tile_pool(name="den", bufs=2)
    out_pool = tc.alloc_tile_pool(name="ot", bufs=3)
    poolsB, tagsB = (ps_mi, ps_mi, ps_mi), ("misc", "misc", "misc")

    o_t = {}
    po_cur = {}
    sp_state = {}

    # two k-halves (kg 0-3 = k tokens 0-1023, kg 4-7 = 1024-2047); the
    # second half's k/v chunks and q chunks 1-2 are produced by filler
    # units inside the first half's ACT-bound window.
    steps = [(qc, h, kg)
             for half in range(2)
             for qc in range(len(QCHUNKS))
             for h in range(HPG)
             for kg in range(half * 4, half * 4 + 4)]
    NH = len(steps) // 2

    def emit_sp(i):
        qc, h, kg = steps[i]
        coff, W = QCHUNKS[qc]
        t, off = h // 2, (h % 2) * HD
        sp = ps_sp.tile([P, KT_GRP * CH], F32, name="sp", tag="sp")
        for j in range(KT_GRP):
            kt = kg * KT_GRP + j
            _mm(nc, sp[:, j * W:(j + 1) * W],
                k_sb[t][off:off + HD, kt * P:(kt + 1) * P],
                q_sb[t][off:off + HD, coff:coff + W], True, True)
        e = e_pool.tile([P, KT_GRP * CH], F16, name="e", tag="e")
        sp_state[i] = (sp, e)

    def emit_exp(i):
        sp, e = sp_state[i]
        qc, h, kg = steps[i]
        W = QCHUNKS[qc][1]
        nc.scalar.activation(e[:, 0:KT_GRP * W], sp[:, 0:KT_GRP * W], AF.Exp)

    def emit_pv(i):
        qc, h, kg = steps[i]
        W = QCHUNKS[qc][1]
        sp, e = sp_state.pop(i)
        if kg % 4 == 0:
            po_cur[qc] = ps_po.tile([HD + 1, CH], F32, name="po", tag="po")
        po = po_cur[qc]
        for j in range(KT_GRP):
            kt = kg * KT_GRP + j
            _mm(nc, po[:, 0:W], v_sb[:, kt, h, :], e[:, j * W:(j + 1) * W],
                kt % 8 == 0, kt % 8 == 7)
        if kg % 4 == 3:
            acc = o_acc[:, qc, h, 0:W]
            if kg < 4:
                nc.vector.tensor_copy(acc, po[:, 0:W])
            else:
                nc.vector.tensor_tensor(acc, acc, po[:, 0:W], OP.add)

    def norm_outproj_units(qc):
        coff, W = QCHUNKS[qc]
        units = []
        d6p = den_pool.tile([HPG, CH], F32, name="d6p", tag="d6p")
        d6r = den_pool.tile([HPG, CH], F16, name="d6r", tag="d6r")
        o_t[qc] = [o_pool.tile([P, CH], F16, name="ot", tag="ot")
                   for _ in range(NT)]

        def recip():
            # gather per-head denominator rows (o_acc row HD) to partitions
            nc.sync.dma_start(d6p[0:HPG, 0:W],
                              o_acc[HD:HD + 1, qc, 0:HPG, 0:W])
            with nc.allow_low_precision(reason="softmax denom recip in f16"):
                nc.vector.reciprocal(d6r[0:HPG, 0:W], d6p[0:HPG, 0:W])
        units.append(recip)
        for t in range(NT):
            def norm(t=t):
                rrep = ps_mi.tile([P, CH], F32, name="nrr", tag="misc")
                _mm(nc, rrep[:, 0:W], bcast_sb[:, t, :], d6r[:, 0:W],
                    True, True)
                nc.vector.tensor_tensor(
                    o_t[qc][t][0:HD, 0:W], o_acc[0:HD, qc, 2 * t, 0:W],
                    rrep[0:HD, 0:W], OP.mult)
                nc.vector.tensor_tensor(
                    o_t[qc][t][HD:P, 0:W], o_acc[0:HD, qc, 2 * t + 1, 0:W],
                    rrep[HD:P, 0:W], OP.mult)
            units.append(norm)
        for m in range(CT):
            def oproj(m=m):
                pp = ps_mi.tile([P, CH], F32, name="opp", tag="misc")
                for t in range(NT):
                    _mm(nc, pp[:, 0:W], wp_sb[:, t, m * P:(m + 1) * P],
                        o_t[qc][t][:, 0:W], t == 0, t == NT - 1)
                ot = out_pool.tile([P, CH], F32, name="oc", tag="oc")
                nc.vector.tensor_copy(ot[:, 0:W], pp[:, 0:W])
                nc.sync.dma_start(outT[m * P:(m + 1) * P, coff:coff + W],
                                  ot[:, 0:W])
            units.append(oproj)
        return units

    def h1_fillers():
        return (qln_units(1, False, poolsB, tagsB)
                + qln_units(2, False, poolsB, tagsB)
                + kchunk_units(2, False, poolsB, tagsB)
                + kchunk_units(3, False, poolsB, tagsB))

    inject = {0: h1_fillers,
              NH + 24: lambda: norm_outproj_units(0),
              NH + 48: lambda: norm_outproj_units(1)}

    # ---------------- emission with software pipelining ----------------
    fillers = []
    emit_sp(0)
    for i in range(len(steps)):
        if i in inject:
            fillers.extend(inject[i]())
        if i + 1 < len(steps):
            emit_sp(i + 1)
        emit_exp(i)
        emit_pv(i)
        if fillers:
            fillers.pop(0)()
    while fillers:
        fillers.pop(0)()
    for unit in norm_outproj_units(len(QCHUNKS) - 1):
        unit()

    for pool in (out_pool, den_pool, o_pool, e_pool, ps_mi, ps_po, ps_sp,
                 xq_pool, xkv_pool, st16, st32, sq_pool,
                 w_pool, bpool, cpool):
        pool.release()


def build_bass():
    nc = bass.Bass(trn_type="TRN2", debug=False, num_devices=NCORES)
    qxT = nc.dram_tensor("qxT", [C, MQ], F16, kind="ExternalInput").ap()
    kvxT = nc.dram_tensor("kvxT", [C, N], F16, kind="ExternalInput").ap()
    wq = nc.dram_tensor("wq", [C, CL], F16, kind="ExternalInput").ap()
    wk = nc.dram_tensor("wk", [C, CL], F16, kind="ExternalInput").ap()
    wv = nc.dram_tensor("wv", [C, CL], F16, kind="ExternalInput").ap()
    wp = nc.dram_tensor("wp", [CL, C], F16, kind="ExternalInput").ap()
    colsel = nc.dram_tensor("colsel", [P, NT, HPG], F16,
                            kind="ExternalInput").ap()
    bcast = nc.dram_tensor("bcast", [HPG, NT, P], F16,
                           kind="ExternalInput").ap()
    outT = nc.dram_tensor("outT", [C, MQ], F32, kind="ExternalOutput").ap()
    aps = (qxT, kvxT, wq, wk, wv, wp, colsel, bcast, outT)
    with _FixedTileContext(nc) as tc:
        _body(tc, aps)
    return nc


def make_in_maps(q_x, kv_x, attn_mask, Wq, Wkv, Wp):
    colsel = np.zeros((P, NT, HPG), np.float16)
    bcast = np.zeros((HPG, NT, P), np.float16)
    for t in range(NT):
        for pp in range(P):
            colsel[pp, t, 2 * t + pp // HD] = 1.0 / HD
            bcast[2 * t + pp // HD, t, pp] = 1.0

    mask = np.asarray(attn_mask, bool)
    in_maps = []
    for core in range(NCORES):
        b, g = core // G, core % G
        sl = slice(g * CL, (g + 1) * CL)
        idx = np.flatnonzero(mask[b])
        cnt = len(idx)
        assert cnt < MQ, f"mask count {cnt} exceeds padded width {MQ}"
        qxT_c = np.zeros((C, MQ), np.float16)
        qxT_c[:, :cnt] = q_x[b][idx].T
        in_maps.append({
            "qxT": qxT_c,
            "kvxT": np.ascontiguousarray(kv_x[b].T.astype(np.float16)),
            "wq": np.ascontiguousarray(Wq[sl].T.astype(np.float16)),
            "wk": np.ascontiguousarray(Wkv[sl].T.astype(np.float16)),
            "wv": np.ascontiguousarray(
                Wkv[C + g * CL:C + (g + 1) * CL].T.astype(np.float16)),
            "wp": np.ascontiguousarray(Wp[:, sl].T.astype(np.float16)),
            "colsel": colsel,
            "bcast": bcast,
        })
    return in_maps


_NC_CACHE = []


def get_nc():
    if not _NC_CACHE:
        _NC_CACHE.append(build_bass())
    return _NC_CACHE[0]


def kernel(q_x, kv_x, attn_mask, Wq, Wkv, qn_w, qn_b, kn_w, kn_b, Wp, bp,
           _profile=None):
    q_x = np.asarray(q_x, np.float32)
    kv_x = np.asarray(kv_x, np.float32)
    attn_mask = np.asarray(attn_mask, bool)
    Wq = np.asarray(Wq, np.float32)
    Wkv = np.asarray(Wkv, np.float32)
    Wp = np.asarray(Wp, np.float32)
    bp = np.asarray(bp, np.float32)
    if not (np.all(np.asarray(qn_w) == 1) and np.all(np.asarray(qn_b) == 0)
            and np.all(np.asarray(kn_w) == 1) and np.all(np.asarray(kn_b) == 0)):
        raise NotImplementedError("kernel specialized to identity q/k norms")

    nc = get_nc()
    in_maps = make_in_maps(q_x, kv_x, attn_mask, Wq, Wkv, Wp)
    res = bass_utils.run_bass_kernel_spmd(
        nc, in_maps, core_ids=list(range(NCORES)))
    if _profile is not None:
        _profile.append(res)
    out = np.empty((B, N, C), np.float32)
    for b in range(B):
        acc = res.results[G * b]["outT"] + res.results[G * b + 1]["outT"]
        idx = np.flatnonzero(attn_mask[b])
        cnt = len(idx)
        out[b, idx] = acc[:, :cnt].T + bp
        out[b, ~attn_mask[b]] = acc[:, cnt] + bp
    return out


# revision 13
# speedup vs baseline: 1.0918x; 1.0918x over previous
"""Cross-attention (B=4, N=2048, C=768, H=12, HD=64) on 8 TRN2 NeuronCores.

Sharding: core = (batch, head_group): 4 batches x 2 groups of 6 heads.
Each core computes its group's Q/K/V projections, per-head-dim LayerNorm,
attention, and a partial output projection; the host sums the two group
partials per batch and adds the bias.

Key optimizations over the fp32 baseline:
 - All matmul operands are fp16 (1 cycle/row on the PE vs 4 for fp32 mode,
   and fast-weight-load applies).  PSUM accumulation stays fp32.
 - Query-token compaction: the reference masks along QUERY rows only, and
   every masked row produces the same output (the uniform average of V,
   since softmax(-1e9 * 1) is uniform).  The host gathers the ~50% unmasked
   tokens per batch, zero-pads to MQ=1152 columns, and scatters back; one
   guaranteed-pad column (q~ = 0 -> scores 0 -> uniform softmax) supplies
   the shared masked-row output.  Attention/exp/out-proj work drops ~2x.
 - Software pipelining: scores (PE) -> exp (ACT) -> PV (PE) run with one
   k-group of lookahead so the ACT exp stream never starves; Q-projection
   and out-projection work is sliced into small "filler" units emitted
   between attention steps to hide it under the ACT-bound window.
 - Softmax denominators come free from a ones-column appended to V; no
   row-max is needed (LN bounds |S| < ~6, exp(S) fits fp16 comfortably).
 - LN scale (HD^-0.5) is folded into the rsqrt via the activation bias:
   exp(-0.5*ln(var+eps) + ln(scale)) = scale * rsqrt(var+eps).
"""

import numpy as np

import concourse.bass as bass
import concourse.mybir as mybir
from concourse import tile
from concourse import bass_utils
from concourse.tile_scheduler import N_PROCS
from concourse.vector_clock import ScopedClock, VectorClock

F32 = mybir.dt.float32
F16 = mybir.dt.float16
AF = mybir.ActivationFunctionType
OP = mybir.AluOpType

B, N, C, H, HD = 4, 2048, 768, 12, 64
G = 2                 # head groups (tensor parallel)
HPG = H // G          # 6 heads per group
CL = HPG * HD         # 384 local channels
P = 128
CH = 512              # token chunk
NCH = N // CH         # 4 k-side chunks
NT = CL // P          # 3 output tiles per group
CT = C // P           # 6 contraction tiles
TT = N // P           # 16 k token tiles
KT_GRP = 2            # k-tiles per exp group
MQ = 1152             # padded compacted q tokens (counts are ~1024+-45)
QCHUNKS = [(0, 512), (512, 512), (1024, 128)]
EPS = 1e-5
SCALE = HD ** -0.5
LNSCALE = float(np.log(SCALE))
NCORES = 8

_nop_ctr = [0]


class _FixedTileContext(tile.TileContext):
    """Workaround for a walrus build that allows at most ONE sync-wait per
    instruction: split multi-wait instructions into single-wait NoOps on the
    same engine, and emit the kernel-tail drain's waits as a nop chain."""

    def _split_multiwait(self, insts):
        out = []
        for inst in insts:
            si = getattr(inst, "sync_info", None)
            waits = list(si.on_wait) if si is not None and si.on_wait else []
            if len(waits) > 1:
                eng = inst.engine
                for w in waits[:-1]:
                    _nop_ctr[0] += 1
                    nop = mybir.InstNoOp(
                        name=f"I-waitsplit-{_nop_ctr[0]}", ins=[], outs=[]
                    )
                    nop.engine = eng
                    nop.sync_info = mybir.SyncInfo(on_wait=[w], on_update=[])
                    self.nc.register_instruction(nop)
                    out.append(nop)
                inst.sync_info = mybir.SyncInfo(
                    on_wait=[waits[-1]], on_update=list(si.on_update)
                )
            out.append(inst)
        return out

    def _lower_ordered_insts(self, ordered):
        ordered = {bb: self._split_multiwait(ins) for bb, ins in ordered.items()}
        super()._lower_ordered_insts(ordered)

    def _drain_and_barrier(self, tick_clock, wait_clock):
        gc = tick_clock.global_clock
        vals = [gc[p] for p in range(N_PROCS)]
        for p in [q for q, v in enumerate(vals) if v > 0]:
            partial = VectorClock(
                [vals[q] if q == p else 0 for q in range(N_PROCS)]
            )
            nop = self.nc.sync.nop(nofuse=True, hint="tail_drain_wait")
            wait_clock.add_sem_waits(nop.ins, ScopedClock({None: partial}))
        self.nc.sync.drain()
        self.nc.all_engine_barrier()
        assert self.sems is not None
        popped = self.nc._tile_sem_poison_stack.pop()
        assert popped is self._sem_poison
        self.nc.clear_and_free_semaphores(list(self.sems.allocated().values()))
        self.nc.all_engine_barrier()


def _mm(nc, out, lhsT, rhs, start, stop):
    nc.tensor.matmul(
        out, lhsT, rhs, start=start, stop=stop, skip_group_check=True
    )


def _body(tc, aps):
    nc = tc.nc
    qxT, kvxT, wq, wk, wv, wp, colsel, bcast, outT = aps

    cpool = tc.alloc_tile_pool(name="consts", bufs=1)
    bpool = tc.alloc_tile_pool(name="big", bufs=1)
    w_pool = tc.alloc_tile_pool(name="wts", bufs=1)

    colsel_sb = cpool.tile([P, NT, HPG], F16, name="colsel", tag="colsel")
    nc.sync.dma_start(colsel_sb[:], colsel[:])
    bcast_sb = cpool.tile([HPG, NT, P], F16, name="bcast", tag="bcast")
    nc.sync.dma_start(bcast_sb[:], bcast[:])
    eps_sb = cpool.tile([HPG, 1], F32, name="eps", tag="eps")
    nc.vector.memset(eps_sb[:], EPS)
    lnq_sb = cpool.tile([HPG, 1], F32, name="lnq", tag="lnq")
    nc.vector.memset(lnq_sb[:], LNSCALE)

    wk_sb = w_pool.tile([P, CT, CL], F16, name="wk", tag="wk")
    nc.sync.dma_start(wk_sb[:], wk.rearrange("(ct p) m -> p ct m", p=P))
    wv_sb = w_pool.tile([P, CT, CL], F16, name="wv", tag="wv")
    nc.sync.dma_start(wv_sb[:], wv.rearrange("(ct p) m -> p ct m", p=P))
    wq_sb = w_pool.tile([P, CT, CL], F16, name="wq", tag="wq")
    wp_sb = w_pool.tile([P, NT, C], F16, name="wp", tag="wp")

    q_sb = [bpool.tile([P, MQ], F16, name=f"q{t}", tag=f"q{t}")
            for t in range(NT)]
    k_sb = [bpool.tile([P, N], F16, name=f"k{t}", tag=f"k{t}")
            for t in range(NT)]
    v_sb = bpool.tile([P, TT, HPG, HD + 1], F16, name="v", tag="v")
    nc.vector.memset(v_sb[:, :, :, HD], 1.0)
    # attention accumulator across the two k-halves (f32, row HD = denom)
    o_acc = bpool.tile([HD + 1, len(QCHUNKS), HPG, CH], F32,
                       name="oacc", tag="oacc")

    sq_pool = tc.alloc_tile_pool(name="sq", bufs=6)
    st32 = tc.alloc_tile_pool(name="st32", bufs=6)
    st16 = tc.alloc_tile_pool(name="st16", bufs=4)
    xkv_pool = tc.alloc_tile_pool(name="xkv", bufs=12)
    xq_pool = tc.alloc_tile_pool(name="xq", bufs=12)

    def chunk_units(xT, w_sb, dst, is_q, coff, W, xpool, pp_pool, st_pool,
                    rr_pool, with_v, use_act, tags):
        """Emission units for projecting one token chunk + per-head-dim LN
        (+ V projection).  use_act routes squares/copies to the Scalar
        engine (good when exp isn't running) vs Vector."""
        tag_pp, tag_st, tag_rr = tags
        st = {"xts": [], "sqs": [None] * NT}
        units = []

        def dmas():
            for ct in range(CT):
                xt = xpool.tile([P, CH], F16, name="xt", tag="xt")
                nc.sync.dma_start(
                    xt[:, 0:W], xT[ct * P:(ct + 1) * P, coff:coff + W])
                st["xts"].append(xt)
        units.append(dmas)
        for t in range(NT):
            def proj(t=t):
                pp = pp_pool.tile([P, CH], F32, name="pp", tag=tag_pp)
                for ct in range(CT):
                    _mm(nc, pp[:, 0:W], w_sb[:, ct, t * P:(t + 1) * P],
                        st["xts"][ct][:, 0:W], ct == 0, ct == CT - 1)
                nc.vector.tensor_copy(dst[t][:, coff:coff + W], pp[:, 0:W])
                sq = sq_pool.tile([P, CH], F16, name="sq", tag="sq")
                if use_act:
                    nc.scalar.activation(sq[:, 0:W], pp[:, 0:W], AF.Square)
                else:
                    nc.vector.tensor_tensor(
                        sq[:, 0:W], dst[t][:, coff:coff + W],
                        dst[t][:, coff:coff + W], OP.mult)
                st["sqs"][t] = sq
            units.append(proj)

        def stats():
            mu_ps = st_pool.tile([HPG, CH], F32, name="mu_ps", tag=tag_st)
            for t in range(NT):
                _mm(nc, mu_ps[:, 0:W], colsel_sb[:, t, :],
                    dst[t][:, coff:coff + W], t == 0, t == NT - 1)
            ms_ps = st_pool.tile([HPG, CH], F32, name="ms_ps", tag=tag_st)
            for t in range(NT):
                _mm(nc, ms_ps[:, 0:W], colsel_sb[:, t, :],
                    st["sqs"][t][:, 0:W], t == 0, t == NT - 1)
            mu = st32.tile([HPG, CH], F32, name="mu", tag="mu")
            nc.vector.tensor_copy(mu[:, 0:W], mu_ps[:, 0:W])
            var = st32.tile([HPG, CH], F32, name="var", tag="var")
            nc.vector.scalar_tensor_tensor(
                var[:, 0:W], mu[:, 0:W], 1.0, mu[:, 0:W], OP.mult, OP.mult)
            nc.vector.tensor_tensor(
                var[:, 0:W], ms_ps[:, 0:W], var[:, 0:W], OP.subtract)
            lnv = st32.tile([HPG, CH], F32, name="lnv", tag="lnv")
            nc.scalar.activation(lnv[:, 0:W], var[:, 0:W], AF.Ln,
                                 bias=eps_sb[:])
            rs = st16.tile([HPG, CH], F16, name="rs", tag="rs")
            if is_q:
                nc.scalar.activation(rs[:, 0:W], lnv[:, 0:W], AF.Exp,
                                     scale=-0.5, bias=lnq_sb[:])
            else:
                nc.scalar.activation(rs[:, 0:W], lnv[:, 0:W], AF.Exp,
                                     scale=-0.5)
            murs = st16.tile([HPG, CH], F16, name="murs", tag="murs")
            nc.vector.scalar_tensor_tensor(
                murs[:, 0:W], mu[:, 0:W], -1.0, rs[:, 0:W],
                OP.mult, OP.mult)
            st["rs"], st["murs"] = rs, murs
        units.append(stats)
        for t in range(NT):
            def apply(t=t):
                rrep = rr_pool.tile([P, CH], F32, name="rrep", tag=tag_rr)
                _mm(nc, rrep[:, 0:W], bcast_sb[:, t, :], st["rs"][:, 0:W],
                    True, True)
                mrep = rr_pool.tile([P, CH], F32, name="mrep", tag=tag_rr)
                _mm(nc, mrep[:, 0:W], bcast_sb[:, t, :], st["murs"][:, 0:W],
                    True, True)
                nc.vector.tensor_tensor(
                    dst[t][:, coff:coff + W], dst[t][:, coff:coff + W],
                    rrep[:, 0:W], OP.mult)
                nc.vector.tensor_tensor(
                    dst[t][:, coff:coff + W], dst[t][:, coff:coff + W],
                    mrep[:, 0:W], OP.add)
            units.append(apply)
        if with_v:
            for tl in range(CH // P):
                def vproj(tl=tl):
                    tt = coff // P + tl
                    vp = pp_pool.tile([P, CH], F32, name="vp", tag=tag_pp)
                    for ct in range(CT):
                        _mm(nc, vp[:, 0:CL],
                            st["xts"][ct][:, tl * P:(tl + 1) * P],
                            wv_sb[:, ct, :], ct == 0, ct == CT - 1)
                    vap = vp[:, 0:CL].rearrange("p (h d) -> p h d", h=HPG)
                    if use_act:
                        nc.scalar.activation(
                            v_sb[:, tt, :, 0:HD], vap, AF.Copy)
                    else:
                        nc.vector.tensor_copy(v_sb[:, tt, :, 0:HD], vap)
                units.append(vproj)
        return units

    # ---------------- pre-phase: k chunks 0-1 + q chunk 0 --------------
    ppA = tc.alloc_tile_pool(name="ppA", bufs=4, space="PSUM")
    stA = tc.alloc_tile_pool(name="stA", bufs=2, space="PSUM")
    rrA = tc.alloc_tile_pool(name="rrA", bufs=2, space="PSUM")

    def kchunk_units(c, use_act, pools, tags):
        return chunk_units(kvxT, wk_sb, k_sb, False, c * CH, CH, xkv_pool,
                           pools[0], pools[1], pools[2], True, use_act, tags)

    def qln_units(qc, use_act, pools, tags):
        coff, W = QCHUNKS[qc]
        return chunk_units(qxT, wq_sb, q_sb, True, coff, W, xq_pool,
                           pools[0], pools[1], pools[2], False, use_act,
                           tags)

    poolsA, tagsA = (ppA, stA, rrA), ("pp", "stp", "rr")
    for u in kchunk_units(0, True, poolsA, tagsA):
        u()
    nc.sync.dma_start(wq_sb[:], wq.rearrange("(ct p) m -> p ct m", p=P))
    nc.sync.dma_start(wp_sb[:], wp.rearrange("(t p) m -> p t m", p=P))
    for u in kchunk_units(1, True, poolsA, tagsA):
        u()
    for u in qln_units(0, True, poolsA, tagsA):
        u()

    for pool in (rrA, stA, ppA):
        pool.release()

    # ---------------- attention pools ----------------------------------
    ps_sp = tc.alloc_tile_pool(name="ps_sp", bufs=2, space="PSUM")
    ps_po = tc.alloc_tile_pool(name="ps_po", bufs=2, space="PSUM")
    ps_mi = tc.alloc_tile_pool(name="ps_mi", bufs=2, space="PSUM")
    e_pool = tc.alloc_tile_pool(name="e", bufs=4)
    o_pool = tc.alloc_tile_pool(name="o", bufs=6)
    den_pool = tc.alloc_tile_pool(name="den", bufs=2)
    out_pool = tc.alloc_tile_pool(name="ot", bufs=3)
    poolsB, tagsB = (ps_mi, ps_mi, ps_mi), ("misc", "misc", "misc")

    o_t = {}
    po_cur = {}
    sp_state = {}

    # Attention runs in three k-quarters (k tokens 0-1023, 1024-1535,
    # 1536-2047).  Quarter boundaries close the psum accumulator into the
    # SBUF o_acc (copy, then adds), which lets k/v chunks 2-3, q chunks
    # 1-2 and the norm/out-projection all run as PE filler INSIDE the
    # ACT-bound exp windows instead of as serial phases.  qc2 (the short
    # 128-token chunk) uses 4 k-tiles per exp step to keep its PE:ACT
    # ratio close to the wide chunks'.
    OPENS = (0, 8, 12)
    CLOSES = (7, 11, 15)

    def q_steps(qc, lo, hi, grp):
        return [(qc, h, list(range(k, k + grp)))
                for h in range(HPG) for k in range(lo, hi, grp)]

    quarters = [
        q_steps(0, 0, 8, 2) + q_steps(1, 0, 8, 2) + q_steps(2, 0, 8, 4),
        q_steps(0, 8, 12, 2) + q_steps(1, 8, 12, 2) + q_steps(2, 8, 12, 4),
        q_steps(0, 12, 16, 2) + q_steps(1, 12, 16, 2)
        + q_steps(2, 12, 16, 4),
    ]
    steps = quarters[0] + quarters[1] + quarters[2]
    Q1 = len(quarters[0])
    Q2 = Q1 + len(quarters[1])

    def emit_sp(i):
        qc, h, kts = steps[i]
        coff, W = QCHUNKS[qc]
        t, off = h // 2, (h % 2) * HD
        sp = ps_sp.tile([P, KT_GRP * CH], F32, name="sp", tag="sp")
        for j, kt in enumerate(kts):
            _mm(nc, sp[:, j * W:(j + 1) * W],
                k_sb[t][off:off + HD, kt * P:(kt + 1) * P],
                q_sb[t][off:off + HD, coff:coff + W], True, True)
        e = e_pool.tile([P, KT_GRP * CH], F16, name="e", tag="e")
        sp_state[i] = (sp, e)

    def emit_exp(i):
        sp, e = sp_state[i]
        qc, h, kts = steps[i]
        W = QCHUNKS[qc][1]
        nc.scalar.activation(e[:, 0:len(kts) * W], sp[:, 0:len(kts) * W],
                             AF.Exp)

    def emit_pv(i):
        qc, h, kts = steps[i]
        W = QCHUNKS[qc][1]
        sp, e = sp_state.pop(i)
        if kts[0] in OPENS:
            po_cur[qc] = ps_po.tile([HD + 1, CH], F32, name="po", tag="po")
        po = po_cur[qc]
        for j, kt in enumerate(kts):
            _mm(nc, po[:, 0:W], v_sb[:, kt, h, :], e[:, j * W:(j + 1) * W],
                kt in OPENS, kt in CLOSES)
        if kts[-1] in CLOSES:
            acc = o_acc[:, qc, h, 0:W]
            if kts[-1] == CLOSES[0]:
                nc.vector.tensor_copy(acc, po[:, 0:W])
            else:
                nc.vector.tensor_tensor(acc, acc, po[:, 0:W], OP.add)

    def norm_outproj_units(qc):
        coff, W = QCHUNKS[qc]
        units = []
        d6p = den_pool.tile([HPG, CH], F32, name="d6p", tag="d6p")
        d6r = den_pool.tile([HPG, CH], F16, name="d6r", tag="d6r")
        o_t[qc] = [o_pool.tile([P, CH], F16, name="ot", tag="ot")
                   for _ in range(NT)]

        def recip():
            # gather per-head denominator rows (o_acc row HD) to partitions
            nc.sync.dma_start(d6p[0:HPG, 0:W],
                              o_acc[HD:HD + 1, qc, 0:HPG, 0:W])
            with nc.allow_low_precision(reason="softmax denom recip in f16"):
                nc.vector.reciprocal(d6r[0:HPG, 0:W], d6p[0:HPG, 0:W])
        units.append(recip)
        for t in range(NT):
            def norm(t=t):
                rrep = ps_mi.tile([P, CH], F32, name="nrr", tag="misc")
                _mm(nc, rrep[:, 0:W], bcast_sb[:, t, :], d6r[:, 0:W],
                    True, True)
                nc.vector.tensor_tensor(
                    o_t[qc][t][0:HD, 0:W], o_acc[0:HD, qc, 2 * t, 0:W],
                    rrep[0:HD, 0:W], OP.mult)
                nc.vector.tensor_tensor(
                    o_t[qc][t][HD:P, 0:W], o_acc[0:HD, qc, 2 * t + 1, 0:W],
                    rrep[HD:P, 0:W], OP.mult)
            units.append(norm)
        for m in range(CT):
            def oproj(m=m):
                pp = ps_mi.tile([P, CH], F32, name="opp", tag="misc")
                for t in range(NT):
                    _mm(nc, pp[:, 0:W], wp_sb[:, t, m * P:(m + 1) * P],
                        o_t[qc][t][:, 0:W], t == 0, t == NT - 1)
                ot = out_pool.tile([P, CH], F32, name="oc", tag="oc")
                nc.vector.tensor_copy(ot[:, 0:W], pp[:, 0:W])
                nc.sync.dma_start(outT[m * P:(m + 1) * P, coff:coff + W],
                                  ot[:, 0:W])
            units.append(oproj)
        return units

    inject = {
        0: lambda: (qln_units(1, False, poolsB, tagsB)
                    + qln_units(2, False, poolsB, tagsB)
                    + kchunk_units(2, False, poolsB, tagsB)),
        Q1: lambda: kchunk_units(3, False, poolsB, tagsB),
        Q2 + 12: lambda: norm_outproj_units(0),
        Q2 + 24: lambda: norm_outproj_units(1),
    }

    # ---------------- emission with software pipelining ----------------
    fillers = []
    emit_sp(0)
    for i in range(len(steps)):
        if i in inject:
            fillers.extend(inject[i]())
        if i + 1 < len(steps):
            emit_sp(i + 1)
        emit_exp(i)
        emit_pv(i)
        if fillers:
            fillers.pop(0)()
    while fillers:
        fillers.pop(0)()
    for unit in norm_outproj_units(len(QCHUNKS) - 1):
        unit()

    for pool in (out_pool, den_pool, o_pool, e_pool, ps_mi, ps_po, ps_sp,
                 xq_pool, xkv_pool, st16, st32, sq_pool,
                 w_pool, bpool, cpool):
        pool.release()


def build_bass():
    nc = bass.Bass(trn_type="TRN2", debug=False, num_devices=NCORES)
    qxT = nc.dram_tensor("qxT", [C, MQ], F16, kind="ExternalInput").ap()
    kvxT = nc.dram_tensor("kvxT", [C, N], F16, kind="ExternalInput").ap()
    wq = nc.dram_tensor("wq", [C, CL], F16, kind="ExternalInput").ap()
    wk = nc.dram_tensor("wk", [C, CL], F16, kind="ExternalInput").ap()
    wv = nc.dram_tensor("wv", [C, CL], F16, kind="ExternalInput").ap()
    wp = nc.dram_tensor("wp", [CL, C], F16, kind="ExternalInput").ap()
    colsel = nc.dram_tensor("colsel", [P, NT, HPG], F16,
                            kind="ExternalInput").ap()
    bcast = nc.dram_tensor("bcast", [HPG, NT, P], F16,
                           kind="ExternalInput").ap()
    outT = nc.dram_tensor("outT", [C, MQ], F32, kind="ExternalOutput").ap()
    aps = (qxT, kvxT, wq, wk, wv, wp, colsel, bcast, outT)
    with _FixedTileContext(nc) as tc:
        _body(tc, aps)
    return nc


def make_in_maps(q_x, kv_x, attn_mask, Wq, Wkv, Wp):
    colsel = np.zeros((P, NT, HPG), np.float16)
    bcast = np.zeros((HPG, NT, P), np.float16)
    for t in range(NT):
        for pp in range(P):
            colsel[pp, t, 2 * t + pp // HD] = 1.0 / HD
            bcast[2 * t + pp // HD, t, pp] = 1.0

    mask = np.asarray(attn_mask, bool)
    in_maps = []
    for core in range(NCORES):
        b, g = core // G, core % G
        sl = slice(g * CL, (g + 1) * CL)
        idx = np.flatnonzero(mask[b])
        cnt = len(idx)
        assert cnt < MQ, f"mask count {cnt} exceeds padded width {MQ}"
        qxT_c = np.zeros((C, MQ), np.float16)
        qxT_c[:, :cnt] = q_x[b][idx].T
        in_maps.append({
            "qxT": qxT_c,
            "kvxT": np.ascontiguousarray(kv_x[b].T.astype(np.float16)),
            "wq": np.ascontiguousarray(Wq[sl].T.astype(np.float16)),
            "wk": np.ascontiguousarray(Wkv[sl].T.astype(np.float16)),
            "wv": np.ascontiguousarray(
                Wkv[C + g * CL:C + (g + 1) * CL].T.astype(np.float16)),
            "wp": np.ascontiguousarray(Wp[:, sl].T.astype(np.float16)),
            "colsel": colsel,
            "bcast": bcast,
        })
    return in_maps


_NC_CACHE = []


def get_nc():
    if not _NC_CACHE:
        _NC_CACHE.append(build_bass())
    return _NC_CACHE[0]


def kernel(q_x, kv_x, attn_mask, Wq, Wkv, qn_w, qn_b, kn_w, kn_b, Wp, bp,
           _profile=None):
    q_x = np.asarray(q_x, np.float32)
    kv_x = np.asarray(kv_x, np.float32)
    attn_mask = np.asarray(attn_mask, bool)
    Wq = np.asarray(Wq, np.float32)
    Wkv = np.asarray(Wkv, np.float32)
    Wp = np.asarray(Wp, np.float32)
    bp = np.asarray(bp, np.float32)
    if not (np.all(np.asarray(qn_w) == 1) and np.all(np.asarray(qn_b) == 0)
            and np.all(np.asarray(kn_w) == 1) and np.all(np.asarray(kn_b) == 0)):
        raise NotImplementedError("kernel specialized to identity q/k norms")

    nc = get_nc()
    in_maps = make_in_maps(q_x, kv_x, attn_mask, Wq, Wkv, Wp)
    res = bass_utils.run_bass_kernel_spmd(
        nc, in_maps, core_ids=list(range(NCORES)))
    if _profile is not None:
        _profile.append(res)
    out = np.empty((B, N, C), np.float32)
    for b in range(B):
        acc = res.results[G * b]["outT"] + res.results[G * b + 1]["outT"]
        idx = np.flatnonzero(attn_mask[b])
        cnt = len(idx)
        out[b, idx] = acc[:, :cnt].T + bp
        out[b, ~attn_mask[b]] = acc[:, cnt] + bp
    return out
